# revision 5
# baseline (speedup 1.0000x reference)
"""Causal multi-head attention (dense transformer block) on 8 Trainium2 NeuronCores.

Problem: x[4, 2048, 1024] -> qkv proj (16 heads x 64) -> causal softmax
attention -> out proj W_out + b_out.

Sharding (hardcoded): data-parallel over the 4 batch elements x
tensor-parallel over 2 head groups (8 heads each) = 8 cores. Each core
computes, for its (batch, head-group):
    Q^T, K^T (transposed layout, scale folded into W_q), V
    per 512-query block: scores S^T = K_h^T.T @ Q_h^T  [keys, queries],
    E = exp(S^T + causal_mask), unnormalized O^T and the softmax
    denominator via one accumulated matmul with a ones-augmented V
    (lhsT = [V_j | 1]), then O^T = O~^T * bcast(1/denominator),
    and a partial output projection with its row-shard of W_out.
The host sums the two partials per batch element and adds b_out.

Self-contained: hardcodes all shapes; builds/compiles the Bass program on
first call and runs it SPMD on cores 0-7 via run_bass_kernel_spmd.
"""

import numpy as np

B, N, DIM = 4, 2048, 1024
HEADS, DH = 16, 64
HPC = HEADS // 2          # heads per core = 8
INNER = HPC * DH          # per-core inner width = 512
QT = 512                  # query-block tokens
KC = 128                  # key chunk
NQB = N // QT             # 4 query blocks
KD = DIM // 128           # 8 contraction chunks over model dim
IC = INNER // 128         # 4 chunks over per-core inner dim
MASK_NEG = -1.0e30

_cache = {}


def _build_program():
    import concourse.bacc as bacc
    import concourse.mybir as mybir
    import concourse.tile as tile

    fp32 = mybir.dt.float32
    Exp = mybir.ActivationFunctionType.Exp

    nc = bacc.Bacc("TRN2", target_bir_lowering=False, debug=False)
    xT = nc.dram_tensor("xT", [DIM, N], fp32, kind="ExternalInput").ap()
    wq = nc.dram_tensor("wq", [DIM, INNER], fp32, kind="ExternalInput").ap()
    wk = nc.dram_tensor("wk", [DIM, INNER], fp32, kind="ExternalInput").ap()
    wv = nc.dram_tensor("wv", [DIM, INNER], fp32, kind="ExternalInput").ap()
    wo = nc.dram_tensor("wo", [INNER, DIM], fp32, kind="ExternalInput").ap()
    cm = nc.dram_tensor("cmask", [QT // KC, KC, QT], fp32, kind="ExternalInput").ap()
    out = nc.dram_tensor("out", [N, DIM], fp32, kind="ExternalOutput").ap()

    with tile.TileContext(nc) as tc:
        with (
            tc.tile_pool(name="persist", bufs=1) as pp,
            tc.tile_pool(name="weights", bufs=1) as wp,
            tc.tile_pool(name="xstream", bufs=10) as xp,
            tc.tile_pool(name="qtpool", bufs=2) as qtp,
            tc.tile_pool(name="otpool", bufs=2) as otp,
            tc.tile_pool(name="epool", bufs=3) as ep,
            tc.tile_pool(name="rcpool", bufs=2) as rcp,
            tc.tile_pool(name="outstage", bufs=2) as osp,
            tc.tile_pool(name="mmpsum", bufs=2, space="PSUM") as mmp,
            tc.tile_pool(name="stpsum", bufs=3, space="PSUM") as stp,
            tc.tile_pool(name="opsum", bufs=3, space="PSUM") as opp,
        ):
            Kt = pp.tile([128, IC, N], fp32, name="Kt")
            Vaug = pp.tile([128, N // KC, HPC, DH + 1], fp32, name="Vaug")
            mask_sb = pp.tile([128, QT // KC, QT], fp32, name="mask_sb")
            wq_sb = wp.tile([128, KD, INNER], fp32, name="wq_sb")
            wo_sb = wp.tile([128, IC, DIM], fp32, name="wo_sb")

            nc.sync.dma_start(wq_sb[:], wq.rearrange("(ko p) m -> p ko m", p=128))
            nc.sync.dma_start(wo_sb[:], wo.rearrange("(ko p) m -> p ko m", p=128))
            nc.sync.dma_start(mask_sb[:], cm.rearrange("r p c -> p r c"))
            nc.vector.memset(Vaug[:, :, :, DH], 1.0)

            # ---- Phase A: K^T (transposed layout) and ones-augmented V ----
            with tc.tile_pool(name="weightsA", bufs=1) as wpa:
                wk_sb = wpa.tile([128, KD, INNER], fp32, name="wk_sb")
                wv_sb = wpa.tile([128, KD, INNER], fp32, name="wv_sb")
                nc.sync.dma_start(wk_sb[:], wk.rearrange("(ko p) m -> p ko m", p=128))
                nc.sync.dma_start(wv_sb[:], wv.rearrange("(ko p) m -> p ko m", p=128))

                for ni in range(N // QT):
                    xa = []
                    for k in range(KD):
                        t = xp.tile([128, QT], fp32, tag="x", name=f"xa_{ni}_{k}")
                        nc.sync.dma_start(
                            t[:], xT[k * 128 : (k + 1) * 128, ni * QT : (ni + 1) * QT]
                        )
                        xa.append(t)
                    for c in range(IC):
                        ps = mmp.tile([128, QT], fp32, tag="mm", name=f"kps_{ni}_{c}")
                        for k in range(KD):
                            nc.tensor.matmul(
                                ps[:],
                                wk_sb[:, k, c * 128 : (c + 1) * 128],
                                xa[k][:],
                                start=(k == 0),
                                stop=(k == KD - 1),
                            )
                        nc.vector.tensor_copy(Kt[:, c, ni * QT : (ni + 1) * QT], ps[:])
                    for t_ in range(QT // KC):
                        ti = ni * (QT // KC) + t_
                        ps = mmp.tile([128, INNER], fp32, tag="mm", name=f"vps_{ti}")
                        for k in range(KD):
                            nc.tensor.matmul(
                                ps[:],
                                xa[k][:, t_ * KC : (t_ + 1) * KC],
                                wv_sb[:, k, :],
                                start=(k == 0),
                                stop=(k == KD - 1),
                            )
                        nc.vector.tensor_copy(
                            Vaug[:, ti, :, 0:DH],
                            ps[:].rearrange("p (h d) -> p h d", h=HPC),
                        )

            # ---- Phase B: per query block: Q^T, attention, out projection ----
            for qi in range(NQB):
                xb = []
                for k in range(KD):
                    t = xp.tile([128, QT], fp32, tag="x", name=f"xb_{qi}_{k}")
                    nc.sync.dma_start(
                        t[:], xT[k * 128 : (k + 1) * 128, qi * QT : (qi + 1) * QT]
                    )
                    xb.append(t)
                Qt = qtp.tile([128, IC, QT], fp32, tag="qt", name=f"qt_{qi}")
                for c in range(IC):
                    ps = mmp.tile([128, QT], fp32, tag="mm", name=f"qps_{qi}_{c}")
                    for k in range(KD):
                        nc.tensor.matmul(
                            ps[:],
                            wq_sb[:, k, c * 128 : (c + 1) * 128],
                            xb[k][:],
                            start=(k == 0),
                            stop=(k == KD - 1),
                        )
                    nc.vector.tensor_copy(Qt[:, c, :], ps[:])

                Ot = otp.tile([128, IC, QT], fp32, tag="ot", name=f"ot_{qi}")
                njc = (qi + 1) * (QT // KC)
                for hp in range(HPC // 2):
                    po = [
                        opp.tile([DH + 1, QT], fp32, tag="o", name=f"po_{qi}_{hp}_{p}")
                        for p in range(2)
                    ]
                    for j in range(njc):
                        for par in range(2):
                            h = 2 * hp + par
                            lo, hi = par * DH, (par + 1) * DH
                            ps = stp.tile(
                                [128, QT], fp32, tag="st", name=f"st_{qi}_{hp}_{j}_{par}"
                            )
                            nc.tensor.matmul(
                                ps[:],
                                Kt[lo:hi, hp, j * KC : (j + 1) * KC],
                                Qt[lo:hi, hp, :],
                                start=True,
                                stop=True,
                            )
                            if j >= qi * (QT // KC):
                                r = j - qi * (QT // KC)
                                nc.vector.tensor_add(
                                    out=ps[:], in0=ps[:], in1=mask_sb[:, r, :]
                                )
                            e = ep.tile(
                                [128, QT], fp32, tag="e", name=f"e_{qi}_{hp}_{j}_{par}"
                            )
                            nc.scalar.activation(e[:], ps[:], Exp)
                            nc.tensor.matmul(
                                po[par][:],
                                Vaug[:, j, h, :],
                                e[:],
                                start=(j == 0),
                                stop=(j == njc - 1),
                            )
                    for par in range(2):
                        lo, hi = par * DH, (par + 1) * DH
                        rc = rcp.tile([1, QT], fp32, tag="rc", name=f"rc_{qi}_{hp}_{par}")
                        nc.vector.reciprocal(rc[:], po[par][DH : DH + 1, :])
                        rbc = rcp.tile(
                            [DH, QT], fp32, tag="rbc", name=f"rbc_{qi}_{hp}_{par}"
                        )
                        nc.gpsimd.partition_broadcast(rbc[:], rc[:])
                        nc.vector.tensor_mul(
                            out=Ot[lo:hi, hp, :], in0=po[par][0:DH, :], in1=rbc[:]
                        )

                for m in range(QT // 128):
                    for nn in range(DIM // 512):
                        ps = mmp.tile([128, 512], fp32, tag="mm", name=f"fps_{qi}_{m}_{nn}")
                        for c in range(IC):
                            nc.tensor.matmul(
                                ps[:],
                                Ot[:, c, m * 128 : (m + 1) * 128],
                                wo_sb[:, c, nn * 512 : (nn + 1) * 512],
                                start=(c == 0),
                                stop=(c == IC - 1),
                            )
                        ost = osp.tile([128, 512], fp32, tag="ost", name=f"ost_{qi}_{m}_{nn}")
                        nc.vector.tensor_copy(ost[:], ps[:])
                        nc.sync.dma_start(
                            out[
                                qi * QT + m * 128 : qi * QT + (m + 1) * 128,
                                nn * 512 : (nn + 1) * 512,
                            ],
                            ost[:],
                        )
    nc.compile()
    return nc


def _get_program():
    if "nc" not in _cache:
        _cache["nc"] = _build_program()
    return _cache["nc"]


def _causal_masks():
    # masks[r, p, c] = 0 if key (r*128 + p) <= query c else -1e30, for the
    # 4 key chunks overlapping the diagonal 512x512 region of a query block.
    r = np.arange(QT // KC)[:, None, None]
    p = np.arange(KC)[None, :, None]
    c = np.arange(QT)[None, None, :]
    return np.where(r * KC + p <= c, 0.0, MASK_NEG).astype(np.float32)


def _make_in_maps(x, W_qkv, W_out):
    scale = DH ** -0.5
    masks = _causal_masks()
    xTs = [np.ascontiguousarray(x[b].T) for b in range(B)]
    in_maps = []
    for core in range(8):
        b, g = core // 2, core % 2
        cols = slice(g * INNER, (g + 1) * INNER)
        in_maps.append(
            {
                "xT": xTs[b],
                "wq": np.ascontiguousarray(W_qkv[:, cols]) * np.float32(scale),
                "wk": np.ascontiguousarray(W_qkv[:, 1024:][:, cols]),
                "wv": np.ascontiguousarray(W_qkv[:, 2048:][:, cols]),
                "wo": np.ascontiguousarray(W_out[g * INNER : (g + 1) * INNER, :]),
                "cmask": masks,
            }
        )
    return in_maps


def _run(inputs, trace=False, trace_cores=None, tmpdir=None):
    from concourse.bass_utils import run_bass_kernel_spmd

    x = np.asarray(inputs["x"], dtype=np.float32)
    W_qkv = np.asarray(inputs["W_qkv"], dtype=np.float32)
    W_out = np.asarray(inputs["W_out"], dtype=np.float32)
    b_out = np.asarray(inputs["b_out"], dtype=np.float32)

    nc = _get_program()
    in_maps = _make_in_maps(x, W_qkv, W_out)
    res = run_bass_kernel_spmd(
        nc,
        in_maps,
        core_ids=list(range(8)),
        trace=trace,
        trace_cores=trace_cores,
        tmpdir=tmpdir,
    )
    outp = np.empty((B, N, DIM), dtype=np.float32)
    for b in range(B):
        outp[b] = res.results[2 * b]["out"] + res.results[2 * b + 1]["out"] + b_out
    return outp, res


def kernel(**inputs):
    outp, _ = _run(inputs, trace=False)
    return outp


# revision 16
# speedup vs baseline: 2.2165x; 2.2165x over previous
"""Causal multi-head attention (dense transformer block) on 8 Trainium2 NeuronCores.

Problem: x[4, 2048, 1024] -> qkv proj (16 heads x 64) -> causal softmax
attention -> out proj W_out + b_out.

Sharding (hardcoded): data-parallel over the 4 batch elements x
tensor-parallel over 2 head groups (8 heads each) = 8 cores. Each core
computes, for its (batch, head-group):
    Q^T, K^T (transposed layout, scale folded into W_q), V
    per 512-query block: scores S^T = K_h^T.T @ Q_h^T  [keys, queries],
    E = exp(S^T + causal_mask), unnormalized O^T and the softmax
    denominator via one accumulated matmul with a ones-augmented V
    (lhsT = [V_j | 1]), then O^T = O~^T * bcast(1/denominator),
    and a partial output projection with its row-shard of W_out.
The host sums the two partials per batch element and adds b_out.

Self-contained: hardcodes all shapes; builds/compiles the Bass program on
first call and runs it SPMD on cores 0-7 via run_bass_kernel_spmd.
"""

import numpy as np

B, N, DIM = 4, 2048, 1024
HEADS, DH = 16, 64
HPC = HEADS // 2          # heads per core = 8
INNER = HPC * DH          # per-core inner width = 512
QT = 512                  # query-block tokens
KC = 128                  # key chunk
NQB = N // QT             # 4 query blocks
KD = DIM // 128           # 8 contraction chunks over model dim
IC = INNER // 128         # 4 chunks over per-core inner dim
MASK_NEG = -1.0e30

_cache = {}


def _build_program():
    import concourse.bacc as bacc
    import concourse.mybir as mybir
    import concourse.tile as tile

    fp32 = mybir.dt.float32
    bf16 = mybir.dt.bfloat16
    Exp = mybir.ActivationFunctionType.Exp

    nc = bacc.Bacc("TRN2", target_bir_lowering=False, debug=False)
    xT = nc.dram_tensor("xT", [DIM, N], bf16, kind="ExternalInput").ap()
    wq = nc.dram_tensor("wq", [DIM, INNER], bf16, kind="ExternalInput").ap()
    wk = nc.dram_tensor("wk", [DIM, INNER], bf16, kind="ExternalInput").ap()
    wv = nc.dram_tensor("wv", [DIM, INNER], bf16, kind="ExternalInput").ap()
    wo = nc.dram_tensor("wo", [INNER, DIM], bf16, kind="ExternalInput").ap()
    cm = nc.dram_tensor("cmask", [QT // KC, KC, QT], fp32, kind="ExternalInput").ap()
    out = nc.dram_tensor("out", [N, DIM], fp32, kind="ExternalOutput").ap()

    with tile.TileContext(nc) as tc:
        with (
            tc.tile_pool(name="persist", bufs=1) as pp,
            tc.tile_pool(name="weights", bufs=1) as wp,
            tc.tile_pool(name="xstream", bufs=12) as xp,
            tc.tile_pool(name="qtpool", bufs=2) as qtp,
            tc.tile_pool(name="otpool", bufs=2) as otp,
            tc.tile_pool(name="epool", bufs=4) as ep,
            tc.tile_pool(name="rcpool", bufs=2) as rcp,
            tc.tile_pool(name="outstage", bufs=3) as osp,
            tc.tile_pool(name="mmpsum", bufs=2, space="PSUM") as mmp,
            tc.tile_pool(name="stpsum", bufs=3, space="PSUM") as stp,
            tc.tile_pool(name="opsum", bufs=3, space="PSUM") as opp,
        ):
            Kt = pp.tile([128, IC, N], bf16, name="Kt")
            Vaug = pp.tile([128, N // KC, HPC, DH + 1], bf16, name="Vaug")
            mask_sb = pp.tile([128, QT // KC, QT], fp32, name="mask_sb")
            wq_sb = wp.tile([128, KD, INNER], bf16, name="wq_sb")
            wo_sb = wp.tile([128, IC, DIM], bf16, name="wo_sb")

            nc.sync.dma_start(wq_sb[:], wq.rearrange("(ko p) m -> p ko m", p=128))
            nc.sync.dma_start(wo_sb[:], wo.rearrange("(ko p) m -> p ko m", p=128))
            nc.sync.dma_start(mask_sb[:], cm.rearrange("r p c -> p r c"))
            nc.vector.memset(Vaug[:, :, :, DH], 1.0)

            # ---- Phase A: K^T (transposed layout) and ones-augmented V ----
            with tc.tile_pool(name="weightsA", bufs=1) as wpa:
                wk_sb = wpa.tile([128, KD, INNER], bf16, name="wk_sb")
                wv_sb = wpa.tile([128, KD, INNER], bf16, name="wv_sb")
                nc.sync.dma_start(wk_sb[:], wk.rearrange("(ko p) m -> p ko m", p=128))
                nc.sync.dma_start(wv_sb[:], wv.rearrange("(ko p) m -> p ko m", p=128))

                for ni in range(N // QT):
                    xa = []
                    for k in range(KD):
                        t = xp.tile([128, QT], bf16, tag="x", name=f"xa_{ni}_{k}")
                        nc.sync.dma_start(
                            t[:], xT[k * 128 : (k + 1) * 128, ni * QT : (ni + 1) * QT]
                        )
                        xa.append(t)
                    for c in range(IC):
                        ps = mmp.tile([128, QT], fp32, tag="mm", name=f"kps_{ni}_{c}")
                        for k in range(KD):
                            nc.tensor.matmul(
                                ps[:],
                                (wk_sb[:, k, c * 128 : (c + 1) * 128]),
                                (xa[k][:]),
                                start=(k == 0),
                                stop=(k == KD - 1),
                            )
                        nc.vector.tensor_copy(Kt[:, c, ni * QT : (ni + 1) * QT], ps[:])
                    for t_ in range(QT // KC):
                        ti = ni * (QT // KC) + t_
                        ps = mmp.tile([128, INNER], fp32, tag="mm", name=f"vps_{ti}")
                        for k in range(KD):
                            nc.tensor.matmul(
                                ps[:],
                                (xa[k][:, t_ * KC : (t_ + 1) * KC]),
                                (wv_sb[:, k, :]),
                                start=(k == 0),
                                stop=(k == KD - 1),
                            )
                        nc.vector.tensor_copy(
                            Vaug[:, ti, :, 0:DH],
                            ps[:].rearrange("p (h d) -> p h d", h=HPC),
                        )

            # ---- Phase B: per query block: Q^T, attention, out projection ----
            for qi in range(NQB):
                xb = []
                for k in range(KD):
                    t = xp.tile([128, QT], bf16, tag="x", name=f"xb_{qi}_{k}")
                    nc.sync.dma_start(
                        t[:], xT[k * 128 : (k + 1) * 128, qi * QT : (qi + 1) * QT]
                    )
                    xb.append(t)
                Qt = qtp.tile([128, IC, QT], bf16, tag="qt", name=f"qt_{qi}")
                for c in range(IC):
                    ps = mmp.tile([128, QT], fp32, tag="mm", name=f"qps_{qi}_{c}")
                    for k in range(KD):
                        nc.tensor.matmul(
                            ps[:],
                            (wq_sb[:, k, c * 128 : (c + 1) * 128]),
                            (xb[k][:]),
                            start=(k == 0),
                            stop=(k == KD - 1),
                        )
                    nc.vector.tensor_copy(Qt[:, c, :], ps[:])

                Ot = otp.tile([128, IC, QT], bf16, tag="ot", name=f"ot_{qi}")
                cs = rcp.tile([HPC, QT], fp32, tag="cs", name=f"cs_{qi}")
                njc = (qi + 1) * (QT // KC)
                for hp in range(HPC // 2):
                    po = [
                        opp.tile([DH + 1, QT], fp32, tag="o", name=f"po_{qi}_{hp}_{p}")
                        for p in range(2)
                    ]
                    for j in range(njc):
                        for par in range(2):
                            h = 2 * hp + par
                            lo, hi = par * DH, (par + 1) * DH
                            ps = stp.tile(
                                [128, QT], fp32, tag="st", name=f"st_{qi}_{hp}_{j}_{par}"
                            )
                            nc.tensor.matmul(
                                ps[:],
                                (Kt[lo:hi, hp, j * KC : (j + 1) * KC]),
                                (Qt[lo:hi, hp, :]),
                                start=True,
                                stop=True,
                            )
                            if j >= qi * (QT // KC):
                                r = j - qi * (QT // KC)
                                nc.vector.tensor_add(
                                    out=ps[:], in0=ps[:], in1=mask_sb[:, r, :]
                                )
                            e = ep.tile(
                                [128, QT], bf16, tag="e", name=f"e_{qi}_{hp}_{j}_{par}"
                            )
                            nc.scalar.activation(e[:], ps[:], Exp)
                            nc.tensor.matmul(
                                po[par][:],
                                (Vaug[:, j, h, :]),
                                (e[:]),
                                start=(j == 0),
                                stop=(j == njc - 1),
                            )
                    for par in range(2):
                        h = 2 * hp + par
                        lo, hi = par * DH, (par + 1) * DH
                        # stage unnormalized O~^T and its denominator row.
                        # Engines can't write a single partition at unaligned
                        # base h, so bounce via a base-0 stage tile + SBUF DMA.
                        nc.vector.tensor_copy(Ot[lo:hi, hp, :], po[par][0:DH, :])
                        stg = rcp.tile([1, QT], fp32, tag="stg", name=f"stg_{qi}_{h}")
                        nc.vector.tensor_copy(stg[:], po[par][DH : DH + 1, :])
                        nc.sync.dma_start(cs[h : h + 1, :], stg[:])

                # one batched reciprocal for all 8 heads, then per-head
                # partition-broadcast + in-place normalize
                rcs = rcp.tile([HPC, QT], fp32, tag="rcs", name=f"rcs_{qi}")
                nc.vector.reciprocal(rcs[:], cs[:])
                for h in range(HPC):
                    hp2, par = h // 2, h % 2
                    lo, hi = par * DH, (par + 1) * DH
                    # partition_broadcast must start at partition 0 on HW
                    # (base-64 output slices produce garbage); broadcast the
                    # full 128 partitions and slice both mul inputs at the
                    # same base so the DVE base-match rule is satisfied.
                    rbs = rcp.tile([1, QT], fp32, tag="rbs", name=f"rbs_{qi}_{h}")
                    nc.sync.dma_start(rbs[:], rcs[h : h + 1, :])
                    rbc = rcp.tile([128, QT], fp32, tag="rbc", name=f"rbc_{qi}_{h}")
                    nc.gpsimd.partition_broadcast(rbc[:], rbs[:])
                    nc.vector.tensor_mul(
                        out=Ot[lo:hi, hp2, :],
                        in0=Ot[lo:hi, hp2, :],
                        in1=rbc[lo:hi, :],
                    )

                for m in range(QT // 128):
                    for nn in range(DIM // 512):
                        ps = mmp.tile([128, 512], fp32, tag="mm", name=f"fps_{qi}_{m}_{nn}")
                        for c in range(IC):
                            nc.tensor.matmul(
                                ps[:],
                                (Ot[:, c, m * 128 : (m + 1) * 128]),
                                (wo_sb[:, c, nn * 512 : (nn + 1) * 512]),
                                start=(c == 0),
                                stop=(c == IC - 1),
                            )
                        ost = osp.tile([128, 512], fp32, tag="ost", name=f"ost_{qi}_{m}_{nn}")
                        nc.vector.tensor_copy(ost[:], ps[:])
                        nc.sync.dma_start(
                            out[
                                qi * QT + m * 128 : qi * QT + (m + 1) * 128,
                                nn * 512 : (nn + 1) * 512,
                            ],
                            ost[:],
                        )
    nc.compile()
    return nc


def _get_program():
    if "nc" not in _cache:
        _cache["nc"] = _build_program()
    return _cache["nc"]


def _causal_masks():
    # masks[r, p, c] = 0 if key (r*128 + p) <= query c else -1e30, for the
    # 4 key chunks overlapping the diagonal 512x512 region of a query block.
    r = np.arange(QT // KC)[:, None, None]
    p = np.arange(KC)[None, :, None]
    c = np.arange(QT)[None, None, :]
    return np.where(r * KC + p <= c, 0.0, MASK_NEG).astype(np.float32)


def _make_in_maps(x, W_qkv, W_out):
    import ml_dtypes

    bf16 = ml_dtypes.bfloat16
    scale = DH ** -0.5
    masks = _causal_masks()
    xTs = [np.ascontiguousarray(x[b].T).astype(bf16) for b in range(B)]
    in_maps = []
    for core in range(8):
        b, g = core // 2, core % 2
        cols = slice(g * INNER, (g + 1) * INNER)
        in_maps.append(
            {
                "xT": xTs[b],
                "wq": (np.ascontiguousarray(W_qkv[:, cols]) * np.float32(scale)).astype(bf16),
                "wk": np.ascontiguousarray(W_qkv[:, 1024:][:, cols]).astype(bf16),
                "wv": np.ascontiguousarray(W_qkv[:, 2048:][:, cols]).astype(bf16),
                "wo": np.ascontiguousarray(W_out[g * INNER : (g + 1) * INNER, :]).astype(bf16),
                "cmask": masks,
            }
        )
    return in_maps


def _run(inputs, trace=False, trace_cores=None, tmpdir=None):
    from concourse.bass_utils import run_bass_kernel_spmd

    x = np.asarray(inputs["x"], dtype=np.float32)
    W_qkv = np.asarray(inputs["W_qkv"], dtype=np.float32)
    W_out = np.asarray(inputs["W_out"], dtype=np.float32)
    b_out = np.asarray(inputs["b_out"], dtype=np.float32)

    nc = _get_program()
    in_maps = _make_in_maps(x, W_qkv, W_out)
    res = run_bass_kernel_spmd(
        nc,
        in_maps,
        core_ids=list(range(8)),
        trace=trace,
        trace_cores=trace_cores,
        tmpdir=tmpdir,
    )
    outp = np.empty((B, N, DIM), dtype=np.float32)
    for b in range(B):
        outp[b] = res.results[2 * b]["out"] + res.results[2 * b + 1]["out"] + b_out
    return outp, res


def kernel(**inputs):
    outp, _ = _run(inputs, trace=False)
    return outp


# revision 18
# speedup vs baseline: 2.6946x; 1.2157x over previous
"""Causal multi-head attention (dense transformer block) on 8 Trainium2 NeuronCores.

Problem: x[4, 2048, 1024] -> qkv proj (16 heads x 64) -> causal softmax
attention -> out proj W_out + b_out.

Sharding (hardcoded): data-parallel over the 4 batch elements x
tensor-parallel over 2 head groups (8 heads each) = 8 cores. Each core
computes, for its (batch, head-group):
    Q^T, K^T (transposed layout, scale folded into W_q), V
    per 512-query block: scores S^T = K_h^T.T @ Q_h^T  [keys, queries],
    E = exp(S^T + causal_mask), unnormalized O^T and the softmax
    denominator via one accumulated matmul with a ones-augmented V
    (lhsT = [V_j | 1]), then O^T = O~^T * bcast(1/denominator),
    and a partial output projection with its row-shard of W_out.
The host sums the two partials per batch element and adds b_out.

Self-contained: hardcodes all shapes; builds/compiles the Bass program on
first call and runs it SPMD on cores 0-7 via run_bass_kernel_spmd.
"""

import numpy as np

B, N, DIM = 4, 2048, 1024
HEADS, DH = 16, 64
HPC = HEADS // 2          # heads per core = 8
INNER = HPC * DH          # per-core inner width = 512
QT = 512                  # query-block tokens
KC = 128                  # key chunk
NQB = N // QT             # 4 query blocks
KD = DIM // 128           # 8 contraction chunks over model dim
IC = INNER // 128         # 4 chunks over per-core inner dim
MASK_NEG = -1.0e30

_cache = {}


def _build_program():
    import concourse.bacc as bacc
    import concourse.mybir as mybir
    import concourse.tile as tile

    fp32 = mybir.dt.float32
    bf16 = mybir.dt.bfloat16
    Exp = mybir.ActivationFunctionType.Exp

    nc = bacc.Bacc("TRN2", target_bir_lowering=False, debug=False)
    xT = nc.dram_tensor("xT", [DIM, N], bf16, kind="ExternalInput").ap()
    wq = nc.dram_tensor("wq", [DIM, INNER], bf16, kind="ExternalInput").ap()
    wk = nc.dram_tensor("wk", [DIM, INNER], bf16, kind="ExternalInput").ap()
    wv = nc.dram_tensor("wv", [DIM, INNER], bf16, kind="ExternalInput").ap()
    wo = nc.dram_tensor("wo", [INNER, DIM], bf16, kind="ExternalInput").ap()
    cm = nc.dram_tensor("cmask", [QT // KC, KC, QT], fp32, kind="ExternalInput").ap()
    out = nc.dram_tensor("out", [N, DIM], fp32, kind="ExternalOutput").ap()

    with tile.TileContext(nc) as tc:
        with (
            tc.tile_pool(name="persist", bufs=1) as pp,
            tc.tile_pool(name="weights", bufs=1) as wp,
            tc.tile_pool(name="xstream", bufs=12) as xp,
            tc.tile_pool(name="qtpool", bufs=2) as qtp,
            tc.tile_pool(name="otpool", bufs=2) as otp,
            tc.tile_pool(name="epool", bufs=4) as ep,
            tc.tile_pool(name="rcpool", bufs=2) as rcp,
            tc.tile_pool(name="outstage", bufs=3) as osp,
            tc.tile_pool(name="mmpsum", bufs=2, space="PSUM") as mmp,
            tc.tile_pool(name="stpsum", bufs=2, space="PSUM") as stp,
            tc.tile_pool(name="opsum", bufs=2, space="PSUM") as opp,
        ):
            Kt = pp.tile([128, IC, N], bf16, name="Kt")
            Vaug = pp.tile([128, N // KC, HPC, DH + 1], bf16, name="Vaug")
            wq_sb = wp.tile([128, KD, INNER], bf16, name="wq_sb")
            wo_sb = wp.tile([128, IC, DIM], bf16, name="wo_sb")

            nc.sync.dma_start(wq_sb[:], wq.rearrange("(ko p) m -> p ko m", p=128))
            nc.sync.dma_start(wo_sb[:], wo.rearrange("(ko p) m -> p ko m", p=128))
            nc.vector.memset(Vaug[:, :, :, DH], 1.0)

            # ---- Phase A: K^T (transposed layout) and ones-augmented V ----
            with tc.tile_pool(name="weightsA", bufs=1) as wpa:
                wk_sb = wpa.tile([128, KD, INNER], bf16, name="wk_sb")
                wv_sb = wpa.tile([128, KD, INNER], bf16, name="wv_sb")
                nc.sync.dma_start(wk_sb[:], wk.rearrange("(ko p) m -> p ko m", p=128))
                nc.sync.dma_start(wv_sb[:], wv.rearrange("(ko p) m -> p ko m", p=128))

                for ni in range(N // QT):
                    xa = []
                    for k in range(KD):
                        t = xp.tile([128, QT], bf16, tag="x", name=f"xa_{ni}_{k}")
                        nc.sync.dma_start(
                            t[:], xT[k * 128 : (k + 1) * 128, ni * QT : (ni + 1) * QT]
                        )
                        xa.append(t)
                    for c in range(IC):
                        ps = mmp.tile([128, QT], fp32, tag="mm", name=f"kps_{ni}_{c}")
                        for k in range(KD):
                            nc.tensor.matmul(
                                ps[:],
                                (wk_sb[:, k, c * 128 : (c + 1) * 128]),
                                (xa[k][:]),
                                start=(k == 0),
                                stop=(k == KD - 1),
                            )
                        nc.vector.tensor_copy(Kt[:, c, ni * QT : (ni + 1) * QT], ps[:])
                    for t_ in range(QT // KC):
                        ti = ni * (QT // KC) + t_
                        ps = mmp.tile([128, INNER], fp32, tag="mm", name=f"vps_{ti}")
                        for k in range(KD):
                            nc.tensor.matmul(
                                ps[:],
                                (xa[k][:, t_ * KC : (t_ + 1) * KC]),
                                (wv_sb[:, k, :]),
                                start=(k == 0),
                                stop=(k == KD - 1),
                            )
                        nc.vector.tensor_copy(
                            Vaug[:, ti, :, 0:DH],
                            ps[:].rearrange("p (h d) -> p h d", h=HPC),
                        )

            # ---- Phase B: per query block: Q^T, attention, out projection ----
            for qi in range(NQB):
                xb = []
                for k in range(KD):
                    t = xp.tile([128, QT], bf16, tag="x", name=f"xb_{qi}_{k}")
                    nc.sync.dma_start(
                        t[:], xT[k * 128 : (k + 1) * 128, qi * QT : (qi + 1) * QT]
                    )
                    xb.append(t)
                Qt = qtp.tile([128, IC, QT], bf16, tag="qt", name=f"qt_{qi}")
                for c in range(IC):
                    ps = mmp.tile([128, QT], fp32, tag="mm", name=f"qps_{qi}_{c}")
                    for k in range(KD):
                        nc.tensor.matmul(
                            ps[:],
                            (wq_sb[:, k, c * 128 : (c + 1) * 128]),
                            (xb[k][:]),
                            start=(k == 0),
                            stop=(k == KD - 1),
                        )
                    nc.vector.tensor_copy(Qt[:, c, :], ps[:])

                Ot = otp.tile([128, IC, QT], bf16, tag="ot", name=f"ot_{qi}")
                cs = rcp.tile([HPC, QT], fp32, tag="cs", name=f"cs_{qi}")
                njc = (qi + 1) * (QT // KC)
                for hp in range(HPC // 2):
                    po = [
                        opp.tile([DH + 1, QT], fp32, tag="o", name=f"po_{qi}_{hp}_{p}")
                        for p in range(2)
                    ]
                    for j in range(njc):
                        # both heads of the pair share one [128, 1024] PSUM
                        # (2 banks) -> a single exp per j
                        ps = stp.tile(
                            [128, 2 * QT], fp32, tag="st", name=f"st_{qi}_{hp}_{j}"
                        )
                        for par in range(2):
                            lo, hi = par * DH, (par + 1) * DH
                            nc.tensor.matmul(
                                ps[:, par * QT : (par + 1) * QT],
                                (Kt[lo:hi, hp, j * KC : (j + 1) * KC]),
                                (Qt[lo:hi, hp, :]),
                                start=True,
                                stop=True,
                            )
                        e = ep.tile(
                            [128, 2 * QT], bf16, tag="e", name=f"e_{qi}_{hp}_{j}"
                        )
                        nc.scalar.activation(e[:], ps[:], Exp)
                        if j >= qi * (QT // KC):
                            # causal zeroing of E on the idle GpSimd engine:
                            # keep where (c - p - 128*r) >= 0, same pattern
                            # for both 512-column head groups
                            r = j - qi * (QT // KC)
                            nc.gpsimd.affine_select(
                                out=e[:].rearrange("p (g c) -> p g c", g=2),
                                in_=e[:].rearrange("p (g c) -> p g c", g=2),
                                compare_op=mybir.AluOpType.is_ge,
                                fill=0.0,
                                base=-(r * KC),
                                channel_multiplier=-1,
                                pattern=[[0, 2], [1, QT]],
                            )
                        for par in range(2):
                            h = 2 * hp + par
                            nc.tensor.matmul(
                                po[par][:],
                                (Vaug[:, j, h, :]),
                                (e[:, par * QT : (par + 1) * QT]),
                                start=(j == 0),
                                stop=(j == njc - 1),
                            )
                    for par in range(2):
                        h = 2 * hp + par
                        lo, hi = par * DH, (par + 1) * DH
                        # stage unnormalized O~^T and its denominator row.
                        # Engines can't write a single partition at unaligned
                        # base h, so bounce via a base-0 stage tile + SBUF DMA.
                        nc.vector.tensor_copy(Ot[lo:hi, hp, :], po[par][0:DH, :])
                        stg = rcp.tile([1, QT], fp32, tag="stg", name=f"stg_{qi}_{h}")
                        nc.vector.tensor_copy(stg[:], po[par][DH : DH + 1, :])
                        nc.sync.dma_start(cs[h : h + 1, :], stg[:])

                # one batched reciprocal for all 8 heads, then per-head
                # partition-broadcast + in-place normalize
                rcs = rcp.tile([HPC, QT], fp32, tag="rcs", name=f"rcs_{qi}")
                nc.vector.reciprocal(rcs[:], cs[:])
                for h in range(HPC):
                    hp2, par = h // 2, h % 2
                    lo, hi = par * DH, (par + 1) * DH
                    # partition_broadcast must start at partition 0 on HW
                    # (base-64 output slices produce garbage); broadcast the
                    # full 128 partitions and slice both mul inputs at the
                    # same base so the DVE base-match rule is satisfied.
                    rbs = rcp.tile([1, QT], fp32, tag="rbs", name=f"rbs_{qi}_{h}")
                    nc.sync.dma_start(rbs[:], rcs[h : h + 1, :])
                    rbc = rcp.tile([128, QT], fp32, tag="rbc", name=f"rbc_{qi}_{h}")
                    nc.gpsimd.partition_broadcast(rbc[:], rbs[:])
                    nc.vector.tensor_mul(
                        out=Ot[lo:hi, hp2, :],
                        in0=Ot[lo:hi, hp2, :],
                        in1=rbc[lo:hi, :],
                    )

                for m in range(QT // 128):
                    for nn in range(DIM // 512):
                        ps = mmp.tile([128, 512], fp32, tag="mm", name=f"fps_{qi}_{m}_{nn}")
                        for c in range(IC):
                            nc.tensor.matmul(
                                ps[:],
                                (Ot[:, c, m * 128 : (m + 1) * 128]),
                                (wo_sb[:, c, nn * 512 : (nn + 1) * 512]),
                                start=(c == 0),
                                stop=(c == IC - 1),
                            )
                        ost = osp.tile([128, 512], fp32, tag="ost", name=f"ost_{qi}_{m}_{nn}")
                        nc.vector.tensor_copy(ost[:], ps[:])
                        nc.sync.dma_start(
                            out[
                                qi * QT + m * 128 : qi * QT + (m + 1) * 128,
                                nn * 512 : (nn + 1) * 512,
                            ],
                            ost[:],
                        )
    nc.compile()
    return nc


def _get_program():
    if "nc" not in _cache:
        _cache["nc"] = _build_program()
    return _cache["nc"]


def _causal_masks():
    # masks[r, p, c] = 0 if key (r*128 + p) <= query c else -1e30, for the
    # 4 key chunks overlapping the diagonal 512x512 region of a query block.
    r = np.arange(QT // KC)[:, None, None]
    p = np.arange(KC)[None, :, None]
    c = np.arange(QT)[None, None, :]
    return np.where(r * KC + p <= c, 0.0, MASK_NEG).astype(np.float32)


def _make_in_maps(x, W_qkv, W_out):
    import ml_dtypes

    bf16 = ml_dtypes.bfloat16
    scale = DH ** -0.5
    masks = _causal_masks()
    xTs = [np.ascontiguousarray(x[b].T).astype(bf16) for b in range(B)]
    in_maps = []
    for core in range(8):
        b, g = core // 2, core % 2
        cols = slice(g * INNER, (g + 1) * INNER)
        in_maps.append(
            {
                "xT": xTs[b],
                "wq": (np.ascontiguousarray(W_qkv[:, cols]) * np.float32(scale)).astype(bf16),
                "wk": np.ascontiguousarray(W_qkv[:, 1024:][:, cols]).astype(bf16),
                "wv": np.ascontiguousarray(W_qkv[:, 2048:][:, cols]).astype(bf16),
                "wo": np.ascontiguousarray(W_out[g * INNER : (g + 1) * INNER, :]).astype(bf16),
                "cmask": masks,
            }
        )
    return in_maps


def _run(inputs, trace=False, trace_cores=None, tmpdir=None):
    from concourse.bass_utils import run_bass_kernel_spmd

    x = np.asarray(inputs["x"], dtype=np.float32)
    W_qkv = np.asarray(inputs["W_qkv"], dtype=np.float32)
    W_out = np.asarray(inputs["W_out"], dtype=np.float32)
    b_out = np.asarray(inputs["b_out"], dtype=np.float32)

    nc = _get_program()
    in_maps = _make_in_maps(x, W_qkv, W_out)
    res = run_bass_kernel_spmd(
        nc,
        in_maps,
        core_ids=list(range(8)),
        trace=trace,
        trace_cores=trace_cores,
        tmpdir=tmpdir,
    )
    outp = np.empty((B, N, DIM), dtype=np.float32)
    for b in range(B):
        outp[b] = res.results[2 * b]["out"] + res.results[2 * b + 1]["out"] + b_out
    return outp, res


def kernel(**inputs):
    outp, _ = _run(inputs, trace=False)
    return outp


# revision 20
# speedup vs baseline: 2.9042x; 1.0778x over previous
"""Causal multi-head attention (dense transformer block) on 8 Trainium2 NeuronCores.

Problem: x[4, 2048, 1024] -> qkv proj (16 heads x 64) -> causal softmax
attention -> out proj W_out + b_out.

Sharding (hardcoded): data-parallel over the 4 batch elements x
tensor-parallel over 2 head groups (8 heads each) = 8 cores. Each core
computes, for its (batch, head-group):
    Q^T, K^T (transposed layout, scale folded into W_q), V
    per 512-query block: scores S^T = K_h^T.T @ Q_h^T  [keys, queries],
    E = exp(S^T + causal_mask), unnormalized O^T and the softmax
    denominator via one accumulated matmul with a ones-augmented V
    (lhsT = [V_j | 1]), then O^T = O~^T * bcast(1/denominator),
    and a partial output projection with its row-shard of W_out.
The host sums the two partials per batch element and adds b_out.

Self-contained: hardcodes all shapes; builds/compiles the Bass program on
first call and runs it SPMD on cores 0-7 via run_bass_kernel_spmd.
"""

import numpy as np

B, N, DIM = 4, 2048, 1024
HEADS, DH = 16, 64
HPC = HEADS // 2          # heads per core = 8
INNER = HPC * DH          # per-core inner width = 512
QT = 512                  # query-block tokens
KC = 128                  # key chunk
NQB = N // QT             # 4 query blocks
KD = DIM // 128           # 8 contraction chunks over model dim
IC = INNER // 128         # 4 chunks over per-core inner dim
MASK_NEG = -1.0e30

_cache = {}


def _build_program():
    import concourse.bacc as bacc
    import concourse.mybir as mybir
    import concourse.tile as tile

    fp32 = mybir.dt.float32
    bf16 = mybir.dt.bfloat16
    Exp = mybir.ActivationFunctionType.Exp

    nc = bacc.Bacc("TRN2", target_bir_lowering=False, debug=False)
    xT = nc.dram_tensor("xT", [DIM, N], bf16, kind="ExternalInput").ap()
    wq = nc.dram_tensor("wq", [DIM, INNER], bf16, kind="ExternalInput").ap()
    wk = nc.dram_tensor("wk", [DIM, INNER], bf16, kind="ExternalInput").ap()
    wv = nc.dram_tensor("wv", [DIM, INNER], bf16, kind="ExternalInput").ap()
    wo = nc.dram_tensor("wo", [INNER, DIM], bf16, kind="ExternalInput").ap()
    cm = nc.dram_tensor("cmask", [QT // KC, KC, QT], fp32, kind="ExternalInput").ap()
    out = nc.dram_tensor("out", [N, DIM], fp32, kind="ExternalOutput").ap()

    with tile.TileContext(nc) as tc:
        with (
            tc.tile_pool(name="persist", bufs=1) as pp,
            tc.tile_pool(name="weights", bufs=1) as wp,
            tc.tile_pool(name="xstream", bufs=12) as xp,
            tc.tile_pool(name="qtpool", bufs=2) as qtp,
            tc.tile_pool(name="otpool", bufs=2) as otp,
            tc.tile_pool(name="epool", bufs=4) as ep,
            tc.tile_pool(name="rcpool", bufs=2) as rcp,
            tc.tile_pool(name="outstage", bufs=3) as osp,
            tc.tile_pool(name="mmpsum", bufs=2, space="PSUM") as mmp,
            tc.tile_pool(name="stpsum", bufs=2, space="PSUM") as stp,
            tc.tile_pool(name="opsum", bufs=2, space="PSUM") as opp,
        ):
            Kt = pp.tile([128, IC, N], bf16, name="Kt")
            Vaug = pp.tile([128, N // KC, HPC, DH + 1], bf16, name="Vaug")
            wq_sb = wp.tile([128, KD, INNER], bf16, name="wq_sb")
            wo_sb = wp.tile([128, IC, DIM], bf16, name="wo_sb")

            nc.sync.dma_start(wq_sb[:], wq.rearrange("(ko p) m -> p ko m", p=128))
            nc.sync.dma_start(wo_sb[:], wo.rearrange("(ko p) m -> p ko m", p=128))
            nc.vector.memset(Vaug[:, :, :, DH], 1.0)

            # ---- Phase A: K^T (transposed layout) and ones-augmented V ----
            with tc.tile_pool(name="weightsA", bufs=1) as wpa:
                wk_sb = wpa.tile([128, KD, INNER], bf16, name="wk_sb")
                wv_sb = wpa.tile([128, KD, INNER], bf16, name="wv_sb")
                nc.sync.dma_start(wk_sb[:], wk.rearrange("(ko p) m -> p ko m", p=128))
                nc.sync.dma_start(wv_sb[:], wv.rearrange("(ko p) m -> p ko m", p=128))

                for ni in range(N // QT):
                    xa = []
                    for k in range(KD):
                        t = xp.tile([128, QT], bf16, tag="x", name=f"xa_{ni}_{k}")
                        nc.sync.dma_start(
                            t[:], xT[k * 128 : (k + 1) * 128, ni * QT : (ni + 1) * QT]
                        )
                        xa.append(t)
                    for c in range(IC):
                        ps = mmp.tile([128, QT], fp32, tag="mm", name=f"kps_{ni}_{c}")
                        for k in range(KD):
                            nc.tensor.matmul(
                                ps[:],
                                (wk_sb[:, k, c * 128 : (c + 1) * 128]),
                                (xa[k][:]),
                                start=(k == 0),
                                stop=(k == KD - 1),
                            )
                        nc.vector.tensor_copy(Kt[:, c, ni * QT : (ni + 1) * QT], ps[:])
                    for t_ in range(QT // KC):
                        ti = ni * (QT // KC) + t_
                        ps = mmp.tile([128, INNER], fp32, tag="mm", name=f"vps_{ti}")
                        for k in range(KD):
                            nc.tensor.matmul(
                                ps[:],
                                (xa[k][:, t_ * KC : (t_ + 1) * KC]),
                                (wv_sb[:, k, :]),
                                start=(k == 0),
                                stop=(k == KD - 1),
                            )
                        nc.vector.tensor_copy(
                            Vaug[:, ti, :, 0:DH],
                            ps[:].rearrange("p (h d) -> p h d", h=HPC),
                        )

            # ---- Phase B: per query block: Q^T, attention, out projection ----
            for qi in range(NQB):
                xb = []
                for k in range(KD):
                    t = xp.tile([128, QT], bf16, tag="x", name=f"xb_{qi}_{k}")
                    nc.sync.dma_start(
                        t[:], xT[k * 128 : (k + 1) * 128, qi * QT : (qi + 1) * QT]
                    )
                    xb.append(t)
                Qt = qtp.tile([128, IC, QT], bf16, tag="qt", name=f"qt_{qi}")
                for c in range(IC):
                    ps = mmp.tile([128, QT], fp32, tag="mm", name=f"qps_{qi}_{c}")
                    for k in range(KD):
                        nc.tensor.matmul(
                            ps[:],
                            (wq_sb[:, k, c * 128 : (c + 1) * 128]),
                            (xb[k][:]),
                            start=(k == 0),
                            stop=(k == KD - 1),
                        )
                    nc.vector.tensor_copy(Qt[:, c, :], ps[:])

                # per-chunk Ot tiles: the output projection's c-th matmul
                # only depends on head pair c, so it can start while later
                # head pairs are still in their attention loop
                Ot = [
                    otp.tile([128, QT], bf16, tag=f"ot{c}", name=f"ot_{qi}_{c}")
                    for c in range(IC)
                ]
                njc = (qi + 1) * (QT // KC)
                for hp in range(HPC // 2):
                    po = [
                        opp.tile([DH + 1, QT], fp32, tag="o", name=f"po_{qi}_{hp}_{p}")
                        for p in range(2)
                    ]
                    for j in range(njc):
                        # both heads of the pair share one [128, 1024] PSUM
                        # (2 banks) -> a single exp per j
                        ps = stp.tile(
                            [128, 2 * QT], fp32, tag="st", name=f"st_{qi}_{hp}_{j}"
                        )
                        for par in range(2):
                            lo, hi = par * DH, (par + 1) * DH
                            nc.tensor.matmul(
                                ps[:, par * QT : (par + 1) * QT],
                                (Kt[lo:hi, hp, j * KC : (j + 1) * KC]),
                                (Qt[lo:hi, hp, :]),
                                start=True,
                                stop=True,
                            )
                        e = ep.tile(
                            [128, 2 * QT], bf16, tag="e", name=f"e_{qi}_{hp}_{j}"
                        )
                        nc.scalar.activation(e[:], ps[:], Exp)
                        if j >= qi * (QT // KC):
                            # causal zeroing of E on the idle GpSimd engine:
                            # keep where (c - p - 128*r) >= 0, same pattern
                            # for both 512-column head groups
                            r = j - qi * (QT // KC)
                            nc.gpsimd.affine_select(
                                out=e[:].rearrange("p (g c) -> p g c", g=2),
                                in_=e[:].rearrange("p (g c) -> p g c", g=2),
                                compare_op=mybir.AluOpType.is_ge,
                                fill=0.0,
                                base=-(r * KC),
                                channel_multiplier=-1,
                                pattern=[[0, 2], [1, QT]],
                            )
                        for par in range(2):
                            h = 2 * hp + par
                            nc.tensor.matmul(
                                po[par][:],
                                (Vaug[:, j, h, :]),
                                (e[:, par * QT : (par + 1) * QT]),
                                start=(j == 0),
                                stop=(j == njc - 1),
                            )
                    # per-head pipelined epilogue: copy O~^T out, fast
                    # reciprocal of the denominator row (18-bit accurate,
                    # far below bf16 noise), broadcast, normalize in place.
                    # partition_broadcast must start at partition 0 on HW
                    # (base-64 output slices produce garbage), so broadcast
                    # the full 128 partitions and slice both mul inputs at
                    # the same base (DVE base-match rule).
                    for par in range(2):
                        h = 2 * hp + par
                        lo, hi = par * DH, (par + 1) * DH
                        nc.vector.tensor_copy(Ot[hp][lo:hi, :], po[par][0:DH, :])
                        # custom-DVE ops read garbage from PSUM on HW: bounce
                        # the denominator row through SBUF first
                        den = rcp.tile([1, QT], fp32, tag="den", name=f"den_{qi}_{h}")
                        nc.vector.tensor_copy(den[:], po[par][DH : DH + 1, :])
                        rc = rcp.tile([1, QT], fp32, tag="rc", name=f"rc_{qi}_{h}")
                        nc.vector.reciprocal_approx_fast(rc[:], den[:])
                        rbc = rcp.tile([128, QT], fp32, tag="rbc", name=f"rbc_{qi}_{h}")
                        nc.gpsimd.partition_broadcast(rbc[:], rc[:])
                        nc.vector.tensor_mul(
                            out=Ot[hp][lo:hi, :],
                            in0=Ot[hp][lo:hi, :],
                            in1=rbc[lo:hi, :],
                        )

                for m in range(QT // 128):
                    for nn in range(DIM // 512):
                        ps = mmp.tile([128, 512], fp32, tag="mm", name=f"fps_{qi}_{m}_{nn}")
                        for c in range(IC):
                            nc.tensor.matmul(
                                ps[:],
                                (Ot[c][:, m * 128 : (m + 1) * 128]),
                                (wo_sb[:, c, nn * 512 : (nn + 1) * 512]),
                                start=(c == 0),
                                stop=(c == IC - 1),
                            )
                        ost = osp.tile([128, 512], fp32, tag="ost", name=f"ost_{qi}_{m}_{nn}")
                        nc.vector.tensor_copy(ost[:], ps[:])
                        nc.sync.dma_start(
                            out[
                                qi * QT + m * 128 : qi * QT + (m + 1) * 128,
                                nn * 512 : (nn + 1) * 512,
                            ],
                            ost[:],
                        )
    nc.compile()
    return nc


def _get_program():
    if "nc" not in _cache:
        _cache["nc"] = _build_program()
    return _cache["nc"]


def _causal_masks():
    # masks[r, p, c] = 0 if key (r*128 + p) <= query c else -1e30, for the
    # 4 key chunks overlapping the diagonal 512x512 region of a query block.
    r = np.arange(QT // KC)[:, None, None]
    p = np.arange(KC)[None, :, None]
    c = np.arange(QT)[None, None, :]
    return np.where(r * KC + p <= c, 0.0, MASK_NEG).astype(np.float32)


def _make_in_maps(x, W_qkv, W_out):
    import ml_dtypes

    bf16 = ml_dtypes.bfloat16
    scale = DH ** -0.5
    masks = _causal_masks()
    xTs = [np.ascontiguousarray(x[b].T).astype(bf16) for b in range(B)]
    in_maps = []
    for core in range(8):
        b, g = core // 2, core % 2
        cols = slice(g * INNER, (g + 1) * INNER)
        in_maps.append(
            {
                "xT": xTs[b],
                "wq": (np.ascontiguousarray(W_qkv[:, cols]) * np.float32(scale)).astype(bf16),
                "wk": np.ascontiguousarray(W_qkv[:, 1024:][:, cols]).astype(bf16),
                "wv": np.ascontiguousarray(W_qkv[:, 2048:][:, cols]).astype(bf16),
                "wo": np.ascontiguousarray(W_out[g * INNER : (g + 1) * INNER, :]).astype(bf16),
                "cmask": masks,
            }
        )
    return in_maps


def _run(inputs, trace=False, trace_cores=None, tmpdir=None):
    from concourse.bass_utils import run_bass_kernel_spmd

    x = np.asarray(inputs["x"], dtype=np.float32)
    W_qkv = np.asarray(inputs["W_qkv"], dtype=np.float32)
    W_out = np.asarray(inputs["W_out"], dtype=np.float32)
    b_out = np.asarray(inputs["b_out"], dtype=np.float32)

    nc = _get_program()
    in_maps = _make_in_maps(x, W_qkv, W_out)
    res = run_bass_kernel_spmd(
        nc,
        in_maps,
        core_ids=list(range(8)),
        trace=trace,
        trace_cores=trace_cores,
        tmpdir=tmpdir,
    )
    outp = np.empty((B, N, DIM), dtype=np.float32)
    for b in range(B):
        outp[b] = res.results[2 * b]["out"] + res.results[2 * b + 1]["out"] + b_out
    return outp, res


def kernel(**inputs):
    outp, _ = _run(inputs, trace=False)
    return outp


# revision 21
# speedup vs baseline: 3.1291x; 1.0774x over previous
"""Causal multi-head attention (dense transformer block) on 8 Trainium2 NeuronCores.

Problem: x[4, 2048, 1024] -> qkv proj (16 heads x 64) -> causal softmax
attention -> out proj W_out + b_out.

Sharding (hardcoded): data-parallel over the 4 batch elements x
tensor-parallel over 2 head groups (8 heads each) = 8 cores. Each core
computes, for its (batch, head-group):
    Q^T, K^T (transposed layout, scale folded into W_q), V
    per 512-query block: scores S^T = K_h^T.T @ Q_h^T  [keys, queries],
    E = exp(S^T + causal_mask), unnormalized O^T and the softmax
    denominator via one accumulated matmul with a ones-augmented V
    (lhsT = [V_j | 1]), then O^T = O~^T * bcast(1/denominator),
    and a partial output projection with its row-shard of W_out.
The host sums the two partials per batch element and adds b_out.

Self-contained: hardcodes all shapes; builds/compiles the Bass program on
first call and runs it SPMD on cores 0-7 via run_bass_kernel_spmd.
"""

import numpy as np

B, N, DIM = 4, 2048, 1024
HEADS, DH = 16, 64
HPC = HEADS // 2          # heads per core = 8
INNER = HPC * DH          # per-core inner width = 512
QT = 512                  # query-block tokens
KC = 128                  # key chunk
NQB = N // QT             # 4 query blocks
KD = DIM // 128           # 8 contraction chunks over model dim
IC = INNER // 128         # 4 chunks over per-core inner dim
MASK_NEG = -1.0e30

_cache = {}


def _build_program():
    import concourse.bacc as bacc
    import concourse.mybir as mybir
    import concourse.tile as tile

    fp32 = mybir.dt.float32
    bf16 = mybir.dt.bfloat16
    Exp = mybir.ActivationFunctionType.Exp

    nc = bacc.Bacc("TRN2", target_bir_lowering=False, debug=False)
    xT = nc.dram_tensor("xT", [DIM, N], bf16, kind="ExternalInput").ap()
    wq = nc.dram_tensor("wq", [DIM, INNER], bf16, kind="ExternalInput").ap()
    wk = nc.dram_tensor("wk", [DIM, INNER], bf16, kind="ExternalInput").ap()
    wv = nc.dram_tensor("wv", [DIM, INNER], bf16, kind="ExternalInput").ap()
    wo = nc.dram_tensor("wo", [INNER, DIM], bf16, kind="ExternalInput").ap()
    cm = nc.dram_tensor("cmask", [QT // KC, KC, QT], fp32, kind="ExternalInput").ap()
    out = nc.dram_tensor("out", [N, DIM], fp32, kind="ExternalOutput").ap()

    with tile.TileContext(nc) as tc:
        with (
            tc.tile_pool(name="persist", bufs=1) as pp,
            tc.tile_pool(name="weights", bufs=1) as wp,
            tc.tile_pool(name="xstream", bufs=12) as xp,
            tc.tile_pool(name="otpool", bufs=2) as otp,
            tc.tile_pool(name="epool", bufs=4) as ep,
            tc.tile_pool(name="rcpool", bufs=2) as rcp,
            tc.tile_pool(name="outstage", bufs=3) as osp,
            tc.tile_pool(name="mmpsum", bufs=2, space="PSUM") as mmp,
            tc.tile_pool(name="stpsum", bufs=2, space="PSUM") as stp,
            tc.tile_pool(name="opsum", bufs=2, space="PSUM") as opp,
        ):
            # Kt/Vaug/Qt split per 512-token block so attention on block qi
            # only waits for blocks <= qi of phase A (tile-granularity deps)
            Kt = [pp.tile([128, IC, QT], bf16, name=f"Kt_{b}") for b in range(N // QT)]
            Vaug = [
                pp.tile([128, QT // KC, HPC, DH + 1], bf16, name=f"Vaug_{b}")
                for b in range(N // QT)
            ]
            Qt = [pp.tile([128, IC, QT], bf16, name=f"Qt_{b}") for b in range(N // QT)]
            wq_sb = wp.tile([128, KD, INNER], bf16, name="wq_sb")
            wo_sb = wp.tile([128, IC, DIM], bf16, name="wo_sb")

            nc.sync.dma_start(wq_sb[:], wq.rearrange("(ko p) m -> p ko m", p=128))
            nc.sync.dma_start(wo_sb[:], wo.rearrange("(ko p) m -> p ko m", p=128))
            for b in range(N // QT):
                nc.vector.memset(Vaug[b][:, :, :, DH], 1.0)

            # ---- Phase A: K^T (transposed layout) and ones-augmented V ----
            with tc.tile_pool(name="weightsA", bufs=1) as wpa:
                wk_sb = wpa.tile([128, KD, INNER], bf16, name="wk_sb")
                wv_sb = wpa.tile([128, KD, INNER], bf16, name="wv_sb")
                nc.sync.dma_start(wk_sb[:], wk.rearrange("(ko p) m -> p ko m", p=128))
                nc.sync.dma_start(wv_sb[:], wv.rearrange("(ko p) m -> p ko m", p=128))

                for ni in range(N // QT):
                    xa = []
                    for k in range(KD):
                        t = xp.tile([128, QT], bf16, tag="x", name=f"xa_{ni}_{k}")
                        nc.sync.dma_start(
                            t[:], xT[k * 128 : (k + 1) * 128, ni * QT : (ni + 1) * QT]
                        )
                        xa.append(t)
                    for c in range(IC):
                        ps = mmp.tile([128, QT], fp32, tag="mm", name=f"kps_{ni}_{c}")
                        for k in range(KD):
                            nc.tensor.matmul(
                                ps[:],
                                (wk_sb[:, k, c * 128 : (c + 1) * 128]),
                                (xa[k][:]),
                                start=(k == 0),
                                stop=(k == KD - 1),
                            )
                        nc.vector.tensor_copy(Kt[ni][:, c, :], ps[:])
                    for t_ in range(QT // KC):
                        ps = mmp.tile([128, INNER], fp32, tag="mm", name=f"vps_{ni}_{t_}")
                        for k in range(KD):
                            nc.tensor.matmul(
                                ps[:],
                                (xa[k][:, t_ * KC : (t_ + 1) * KC]),
                                (wv_sb[:, k, :]),
                                start=(k == 0),
                                stop=(k == KD - 1),
                            )
                        nc.vector.tensor_copy(
                            Vaug[ni][:, t_, :, 0:DH],
                            ps[:].rearrange("p (h d) -> p h d", h=HPC),
                        )
                    for c in range(IC):
                        ps = mmp.tile([128, QT], fp32, tag="mm", name=f"qps_{ni}_{c}")
                        for k in range(KD):
                            nc.tensor.matmul(
                                ps[:],
                                (wq_sb[:, k, c * 128 : (c + 1) * 128]),
                                (xa[k][:]),
                                start=(k == 0),
                                stop=(k == KD - 1),
                            )
                        nc.vector.tensor_copy(Qt[ni][:, c, :], ps[:])

            # ---- Phase B: per query block: attention + out projection ----
            for qi in range(NQB):
                # per-chunk Ot tiles: the output projection's c-th matmul
                # only depends on head pair c, so it can start while later
                # head pairs are still in their attention loop
                Ot = [
                    otp.tile([128, QT], bf16, tag=f"ot{c}", name=f"ot_{qi}_{c}")
                    for c in range(IC)
                ]
                njc = (qi + 1) * (QT // KC)
                for hp in range(HPC // 2):
                    po = [
                        opp.tile([DH + 1, QT], fp32, tag="o", name=f"po_{qi}_{hp}_{p}")
                        for p in range(2)
                    ]
                    for j in range(njc):
                        # both heads of the pair share one [128, 1024] PSUM
                        # (2 banks) -> a single exp per j
                        ps = stp.tile(
                            [128, 2 * QT], fp32, tag="st", name=f"st_{qi}_{hp}_{j}"
                        )
                        for par in range(2):
                            lo, hi = par * DH, (par + 1) * DH
                            nc.tensor.matmul(
                                ps[:, par * QT : (par + 1) * QT],
                                (Kt[j // (QT // KC)][
                                    lo:hi, hp, (j % (QT // KC)) * KC : (j % (QT // KC) + 1) * KC
                                ]),
                                (Qt[qi][lo:hi, hp, :]),
                                start=True,
                                stop=True,
                            )
                        e = ep.tile(
                            [128, 2 * QT], bf16, tag="e", name=f"e_{qi}_{hp}_{j}"
                        )
                        nc.scalar.activation(e[:], ps[:], Exp)
                        if j >= qi * (QT // KC):
                            # causal zeroing of E on the idle GpSimd engine:
                            # keep where (c - p - 128*r) >= 0, same pattern
                            # for both 512-column head groups
                            r = j - qi * (QT // KC)
                            nc.gpsimd.affine_select(
                                out=e[:].rearrange("p (g c) -> p g c", g=2),
                                in_=e[:].rearrange("p (g c) -> p g c", g=2),
                                compare_op=mybir.AluOpType.is_ge,
                                fill=0.0,
                                base=-(r * KC),
                                channel_multiplier=-1,
                                pattern=[[0, 2], [1, QT]],
                            )
                        for par in range(2):
                            h = 2 * hp + par
                            nc.tensor.matmul(
                                po[par][:],
                                (Vaug[j // (QT // KC)][:, j % (QT // KC), h, :]),
                                (e[:, par * QT : (par + 1) * QT]),
                                start=(j == 0),
                                stop=(j == njc - 1),
                            )
                    # per-head pipelined epilogue: copy O~^T out, fast
                    # reciprocal of the denominator row (18-bit accurate,
                    # far below bf16 noise), broadcast, normalize in place.
                    # partition_broadcast must start at partition 0 on HW
                    # (base-64 output slices produce garbage), so broadcast
                    # the full 128 partitions and slice both mul inputs at
                    # the same base (DVE base-match rule).
                    for par in range(2):
                        h = 2 * hp + par
                        lo, hi = par * DH, (par + 1) * DH
                        nc.vector.tensor_copy(Ot[hp][lo:hi, :], po[par][0:DH, :])
                        # custom-DVE ops read garbage from PSUM on HW: bounce
                        # the denominator row through SBUF first
                        den = rcp.tile([1, QT], fp32, tag="den", name=f"den_{qi}_{h}")
                        nc.vector.tensor_copy(den[:], po[par][DH : DH + 1, :])
                        rc = rcp.tile([1, QT], fp32, tag="rc", name=f"rc_{qi}_{h}")
                        nc.vector.reciprocal_approx_fast(rc[:], den[:])
                        rbc = rcp.tile([128, QT], fp32, tag="rbc", name=f"rbc_{qi}_{h}")
                        nc.gpsimd.partition_broadcast(rbc[:], rc[:])
                        nc.vector.tensor_mul(
                            out=Ot[hp][lo:hi, :],
                            in0=Ot[hp][lo:hi, :],
                            in1=rbc[lo:hi, :],
                        )

                for m in range(QT // 128):
                    for nn in range(DIM // 512):
                        ps = mmp.tile([128, 512], fp32, tag="mm", name=f"fps_{qi}_{m}_{nn}")
                        for c in range(IC):
                            nc.tensor.matmul(
                                ps[:],
                                (Ot[c][:, m * 128 : (m + 1) * 128]),
                                (wo_sb[:, c, nn * 512 : (nn + 1) * 512]),
                                start=(c == 0),
                                stop=(c == IC - 1),
                            )
                        ost = osp.tile([128, 512], fp32, tag="ost", name=f"ost_{qi}_{m}_{nn}")
                        nc.vector.tensor_copy(ost[:], ps[:])
                        nc.sync.dma_start(
                            out[
                                qi * QT + m * 128 : qi * QT + (m + 1) * 128,
                                nn * 512 : (nn + 1) * 512,
                            ],
                            ost[:],
                        )
    nc.compile()
    return nc


def _get_program():
    if "nc" not in _cache:
        _cache["nc"] = _build_program()
    return _cache["nc"]


def _causal_masks():
    # masks[r, p, c] = 0 if key (r*128 + p) <= query c else -1e30, for the
    # 4 key chunks overlapping the diagonal 512x512 region of a query block.
    r = np.arange(QT // KC)[:, None, None]
    p = np.arange(KC)[None, :, None]
    c = np.arange(QT)[None, None, :]
    return np.where(r * KC + p <= c, 0.0, MASK_NEG).astype(np.float32)


def _make_in_maps(x, W_qkv, W_out):
    import ml_dtypes

    bf16 = ml_dtypes.bfloat16
    scale = DH ** -0.5
    masks = _causal_masks()
    xTs = [np.ascontiguousarray(x[b].T).astype(bf16) for b in range(B)]
    in_maps = []
    for core in range(8):
        b, g = core // 2, core % 2
        cols = slice(g * INNER, (g + 1) * INNER)
        in_maps.append(
            {
                "xT": xTs[b],
                "wq": (np.ascontiguousarray(W_qkv[:, cols]) * np.float32(scale)).astype(bf16),
                "wk": np.ascontiguousarray(W_qkv[:, 1024:][:, cols]).astype(bf16),
                "wv": np.ascontiguousarray(W_qkv[:, 2048:][:, cols]).astype(bf16),
                "wo": np.ascontiguousarray(W_out[g * INNER : (g + 1) * INNER, :]).astype(bf16),
                "cmask": masks,
            }
        )
    return in_maps


def _run(inputs, trace=False, trace_cores=None, tmpdir=None):
    from concourse.bass_utils import run_bass_kernel_spmd

    x = np.asarray(inputs["x"], dtype=np.float32)
    W_qkv = np.asarray(inputs["W_qkv"], dtype=np.float32)
    W_out = np.asarray(inputs["W_out"], dtype=np.float32)
    b_out = np.asarray(inputs["b_out"], dtype=np.float32)

    nc = _get_program()
    in_maps = _make_in_maps(x, W_qkv, W_out)
    res = run_bass_kernel_spmd(
        nc,
        in_maps,
        core_ids=list(range(8)),
        trace=trace,
        trace_cores=trace_cores,
        tmpdir=tmpdir,
    )
    outp = np.empty((B, N, DIM), dtype=np.float32)
    for b in range(B):
        outp[b] = res.results[2 * b]["out"] + res.results[2 * b + 1]["out"] + b_out
    return outp, res


def kernel(**inputs):
    outp, _ = _run(inputs, trace=False)
    return outp
